# revision 4
# baseline (speedup 1.0000x reference)
# Bass/Trainium2 kernel for nn_M2R_25778393710941 (loss_fn).
#
# reference:
#   proj_j = Mj @ W.T ; proj_i = Mi @ W.T            [B, K]
#   pos = einsum('bk,bk->b', proj_j, r[:, rp].T)
#   neg = einsum('bk,bk->b', proj_i, r[:, ri].T)
#   loss = relu(pos - neg + 1).mean()
#
# Shapes: B=4096, NV=16384, NR=10000, K=128.
#
# Strategy (8 cores = 4 batch shards x 2 NV shards; BS=1024 rows and
# NVS=8192 contraction per core):
#   - Hybrid sharding halves the replicated-W HBM traffic vs pure
#     batch-parallel (1 MB vs 2 MB per core); each core computes partial
#     margins over its NV half and the host sums the two halves per batch
#     shard. Per-core HBM traffic: 16.78 MB (M streams) + 1 MB (W) +
#     0.25 MB (r gathers) ~= 18 MB -> ~44 us at the ~410 GB/s per-core
#     DMA rate, which is the wall this kernel sits against.
#   - Host: cast M shards to fp8e4m3 and pack as [p, k, b] (k = 128-row
#     contraction block) so every DMA reads long contiguous per-partition
#     runs; pack W (scaled by K, lossless) to WT[p, k*128+m] = K*W[m, nvs
#     + k*128+p]; gather r columns as K*r[:, idx] in fp8 (margins come out
#     scaled by K^2; the host divides it back out - the device epilogue
#     then needs no scale op).
#   - Device: projT[kw, b] += WT_blk.T @ MT_blk accumulated over the 64
#     nv-blocks into PSUM via fp8 DoubleRow matmuls, in two 512-column
#     halves per matrix (PSUM bank = 512 fp32); one Ldweights serves the
#     4 matmuls (neg0/neg1/pos0/pos1) of a block pair after dedup.
#     Epilogue: u = neg*riT on Pool and t = pos*rpT on DVE in parallel,
#     +/-ones column-sum matmuls into ps_d, engine-parallel PSUM->SBUF
#     copies, DMA out the K^2-scaled partial margins. Host applies
#     /K^2, +1, relu, mean and the cross-NV-shard sum.
#   - Filler matmuls on scratch data pad the PE duty cycle during
#     DMA-limited stretches so the HW activity monitor keeps the PE clock
#     at 2.4 GHz (it drops to 1.2 GHz when PE looks idle, which would
#     make the PE the critical path at the tail of the stream).
import os
import sys

import numpy as np
import ml_dtypes

B, NV, NR, K = 4096, 16384, 10000, 128
NBSH = 4                  # batch shards
NNSH = 2                  # NV shards
NCORES = NBSH * NNSH
BS = B // NBSH            # 1024 batch rows per core
NVS = NV // NNSH          # 8192 contraction per core
P = 128                   # partition dim / nv-block size
NBLK = NVS // P           # 64 contraction blocks
HB = 512                  # PSUM column half (one fp32 bank)
# nv-blocks per SBUF buffer chunk: small leading chunks prime the pipeline
# fast, big middle chunks amortize, small tail chunks cut the final PE burst.
CHUNKS = [2, 2, 4, 8, 16, 16, 8, 4, 2, 2]
assert sum(CHUNKS) == NBLK

_NP_DT = {
    "bfloat16": np.dtype(ml_dtypes.bfloat16),
    "float8e4": np.dtype(ml_dtypes.float8_e4m3),
    "float32": np.dtype(np.float32),
}

_NC = None                # cached compiled Bass program
LAST_RESULTS = None       # stashed BassKernelResults for test.py introspection


def _build_bass():
    import concourse.bacc as bacc
    import concourse.mybir as mybir
    import concourse.tile as tile

    mdt = mybir.dt.float8e4
    f32 = mybir.dt.float32
    bf16 = mybir.dt.bfloat16

    nc = bacc.Bacc(
        "TRN2",
        target_bir_lowering=False,
        debug=False,
        enable_asserts=False,
        num_devices=NCORES,
    )

    # M shards host-packed to [p, k, b] so chunk DMAs read long contiguous
    # per-partition runs (ch*BS bytes) instead of strided segments.
    mjt_d = nc.dram_tensor("mjt", [P, NBLK, BS], mdt, kind="ExternalInput")
    mit_d = nc.dram_tensor("mit", [P, NBLK, BS], mdt, kind="ExternalInput")
    wt_d = nc.dram_tensor("wt", [P, NVS], mdt, kind="ExternalInput")
    rpt_d = nc.dram_tensor("rpt", [P, BS], mdt, kind="ExternalInput")
    rit_d = nc.dram_tensor("rit", [P, BS], mdt, kind="ExternalInput")
    losses_d = nc.dram_tensor("losses", [1, BS], f32, kind="ExternalOutput")
    ones_d = nc.inline_tensor(
        np.ones((P, 1), ml_dtypes.bfloat16), name="ones_c"
    )
    nones_d = nc.inline_tensor(
        np.full((P, 1), -1.0, ml_dtypes.bfloat16), name="nones_c"
    )

    with tile.TileContext(nc) as tc:
        with (
            tc.tile_pool(name="wt", bufs=1) as wt_pool,
            tc.tile_pool(name="m", bufs=5) as m_pool,
            tc.tile_pool(name="consts", bufs=1) as c_pool,
            tc.tile_pool(name="ep", bufs=1) as ep_pool,
            tc.tile_pool(name="ps", bufs=1, space="PSUM") as ps_pool,
        ):
            # Resident packed W.T: the slice the first chunk needs rides the
            # fast Sync queue; the rest prefetches on the GpSimd queue in the
            # background, off the hot M streams.
            wt_sb = wt_pool.tile([P, NVS], mdt)
            nc.sync.dma_start(
                out=wt_sb[:, : CHUNKS[0] * P], in_=wt_d[:, : CHUNKS[0] * P]
            )
            nc.gpsimd.dma_start(
                out=wt_sb[:, CHUNKS[0] * P :], in_=wt_d[:, CHUNKS[0] * P :]
            )

            rpt_sb = c_pool.tile([P, BS], mdt, tag="rpt")
            nc.gpsimd.dma_start(out=rpt_sb[:], in_=rpt_d[:])
            rit_sb = c_pool.tile([P, BS], mdt, tag="rit")
            nc.gpsimd.dma_start(out=rit_sb[:], in_=rit_d[:])
            ones_sb = c_pool.tile([P, 1], bf16, tag="ones")
            nc.gpsimd.dma_start(out=ones_sb[:], in_=ones_d[:])
            nones_sb = c_pool.tile([P, 1], bf16, tag="nones")
            nc.gpsimd.dma_start(out=nones_sb[:], in_=nones_d[:])

            ps_pos = [ps_pool.tile([P, HB], f32, tag=f"pos{h}", name=f"ps_pos{h}") for h in (0, 1)]
            ps_neg = [ps_pool.tile([P, HB], f32, tag=f"neg{h}", name=f"ps_neg{h}") for h in (0, 1)]

            # Scratch operands for HAM-warmth filler matmuls: no data deps
            # beyond their memsets, so they slot into PE idle gaps and keep
            # the activity monitor from demoting the PE clock while DMA is
            # the limiter.
            wsc_sb = c_pool.tile([P, 1], mdt, tag="wsc")
            nc.vector.memset(wsc_sb[:], 1.0)
            xsc_sb = c_pool.tile([P, HB], mdt, tag="xsc")
            nc.vector.memset(xsc_sb[:], 0.125)
            ps_warm = ps_pool.tile([1, HB], f32, tag="warm")

            def fill(n):
                for _ in range(n):
                    nc.tensor.matmul(
                        ps_warm[:], wsc_sb[:], xsc_sb[:], start=True, stop=True
                    )

            blk0 = 0
            for c, ch in enumerate(CHUNKS):
                # Split each chunk's transfer into <=4-block DMAs so matmuls
                # can start on the first sub-slice while the rest streams in
                # (Tile tracks sub-tile ranges), keeping PE idle gaps short.
                mj_sb = m_pool.tile([P, ch, BS], mdt, tag="mj")
                mi_sb = m_pool.tile([P, ch, BS], mdt, tag="mi")
                for s0 in range(0, ch, 4):
                    w = min(4, ch - s0)
                    nc.sync.dma_start(
                        out=mj_sb[:, s0 : s0 + w, :],
                        in_=mjt_d[:, blk0 + s0 : blk0 + s0 + w, :],
                    )
                    nc.scalar.dma_start(
                        out=mi_sb[:, s0 : s0 + w, :],
                        in_=mit_d[:, blk0 + s0 : blk0 + s0 + w, :],
                    )
                # DoubleRow: one matmul consumes two contraction blocks —
                # lhsT [K, 2, M], rhs [K, 2, N] -> out += W0.T@X0 + W1.T@X1.
                # neg before pos so the neg PSUM completes first and its
                # epilogue multiply (on Pool) overlaps the last pos matmuls.
                for k in range(0, ch, 2):
                    kk = blk0 + k
                    wpair = wt_sb[:, kk * P : (kk + 2) * P].rearrange(
                        "p (two m) -> p two m", two=2
                    )
                    for h in (0, 1):
                        nc.tensor.matmul(
                            ps_neg[h][:],
                            wpair,
                            mi_sb[:, k : k + 2, h * HB : (h + 1) * HB],
                            start=(kk == 0),
                            stop=(kk == NBLK - 2),
                            perf_mode=mybir.MatmulPerfMode.DoubleRow,
                        )
                    for h in (0, 1):
                        nc.tensor.matmul(
                            ps_pos[h][:],
                            wpair,
                            mj_sb[:, k : k + 2, h * HB : (h + 1) * HB],
                            start=(kk == 0),
                            stop=(kk == NBLK - 2),
                            perf_mode=mybir.MatmulPerfMode.DoubleRow,
                        )
                fill(max(2, ch // 2))
                blk0 += ch

            # A few more fillers bridge the TT-mult latency so the HAM does
            # not demote the clock before the ps_d column-sum matmuls.
            fill(3)

            # All four PSUM multiplies run on DVE (only DVE/Act can read
            # PSUM, and Act has no tensor_tensor): u's first — the neg
            # PSUMs complete before the last pos matmuls, so the u
            # multiplies overlap the stream tail.
            u_sb = [ep_pool.tile([P, HB], bf16, tag=f"u{h}", name=f"u_sb{h}") for h in (0, 1)]
            t_sb = [ep_pool.tile([P, HB], bf16, tag=f"t{h}", name=f"t_sb{h}") for h in (0, 1)]
            for h in (0, 1):
                nc.vector.tensor_tensor(
                    out=u_sb[h][:],
                    in0=ps_neg[h][:],
                    in1=rit_sb[:, h * HB : (h + 1) * HB],
                    op=mybir.AluOpType.mult,
                )
            for h in (0, 1):
                nc.vector.tensor_tensor(
                    out=t_sb[h][:],
                    in0=ps_pos[h][:],
                    in1=rpt_sb[:, h * HB : (h + 1) * HB],
                    op=mybir.AluOpType.mult,
                )

            # Column-sum over the partition dim via +/-ones matmuls: both u
            # matmuls first (their inputs land first), then t0, then t1 so
            # the half-0 copy can start while half 1 is still summing.
            ps_d = [ps_pool.tile([1, HB], f32, tag=f"d{h}", name=f"ps_d{h}") for h in (0, 1)]
            for h in (0, 1):
                nc.tensor.matmul(
                    ps_d[h][:], nones_sb[:], u_sb[h][:], start=True, stop=False
                )
            for h in (0, 1):
                nc.tensor.matmul(
                    ps_d[h][:], ones_sb[:], t_sb[h][:], start=False, stop=True
                )

            # PSUM -> SBUF copies on two engines in parallel (single-partition
            # ops are lane-limited, so splitting halves the critical path),
            # then one DMA of the K^2-scaled partial margins. The (/K^2, +1,
            # relu, mean) tail runs on the host.
            losses_sb = ep_pool.tile([1, BS], f32, tag="losses")
            nc.vector.tensor_scalar_mul(losses_sb[:, :HB], ps_d[0][:], 1.0)
            nc.scalar.copy(out=losses_sb[:, HB:], in_=ps_d[1][:])
            nc.sync.dma_start(out=losses_d[:], in_=losses_sb[:])

    _dedup_ldweights(nc, mybir)
    nc.compile()
    return nc


def _dedup_ldweights(nc, mybir):
    """Tile lowering emits a standalone Ldweights before every Matmult, even
    when consecutive matmuls share the same stationary operand (our 4-matmul
    block-pair groups). The PE keeps weights loaded across matmuls, so drop a
    Ldweights that exactly repeats the previous one (only Matmults in
    between, no sync attached). Cuts PE weight-load traffic 4x."""
    removed = 0
    for blk in nc.m.functions[0].blocks:
        insts = blk.instructions
        last_key = None
        to_remove = []
        for inst in insts:
            if inst.opcode == "Ldweights":
                key = (str(inst.ins), str(getattr(inst, "perf_mode", None)))
                si = inst.sync_info
                has_sync = si is not None and (
                    list(si.on_wait) or list(si.on_update)
                )
                if key == last_key and not has_sync:
                    to_remove.append(inst)
                else:
                    last_key = key
            elif inst.opcode == "Matmult":
                pass  # stationary weights survive matmuls
            elif inst.engine == mybir.EngineType.PE:
                last_key = None
        for inst in to_remove:
            insts.remove(inst)
        removed += len(to_remove)


def _get_nc():
    global _NC
    if _NC is None:
        _NC = _build_bass()
    return _NC


def _prep_inputs(Mi, Mj, ri, rp, W, r):
    Mi = np.asarray(Mi, dtype=np.float32)
    Mj = np.asarray(Mj, dtype=np.float32)
    ri = np.asarray(ri)
    rp = np.asarray(rp)
    W = np.asarray(W, dtype=np.float32)
    r = np.asarray(r, dtype=np.float32)

    mdt = _NP_DT["float8e4"]

    # WT_n[p, k*P + m] = K * W[m, n*NVS + k*P + p] (contraction block k
    # natural on partitions; the K pre-scale keeps fp8 W at unit variance
    # and is divided back out on the host).
    wts = []
    for n in range(NNSH):
        wsl = (W[:, n * NVS : (n + 1) * NVS] * np.float32(K))
        wts.append(
            np.ascontiguousarray(
                wsl.reshape(K, NBLK, P).transpose(2, 1, 0).reshape(P, NVS)
            ).astype(mdt)
        )

    rpt = (r[:, rp] * np.float32(K)).astype(mdt)  # [K, B] at unit variance
    rit = (r[:, ri] * np.float32(K)).astype(mdt)

    in_maps = []
    for bi in range(NBSH):
        sl = slice(bi * BS, (bi + 1) * BS)
        for n in range(NNSH):
            def pack(M):
                # [BS, NVS] -> [NVS, BS] cast -> [p, k, b] contiguous
                t = M[sl, n * NVS : (n + 1) * NVS].T.astype(mdt, order="C")
                return np.ascontiguousarray(
                    t.reshape(NBLK, P, BS).transpose(1, 0, 2)
                )

            in_maps.append(
                {
                    "mjt": pack(Mj),
                    "mit": pack(Mi),
                    "wt": wts[n],
                    "rpt": np.ascontiguousarray(rpt[:, sl]),
                    "rit": np.ascontiguousarray(rit[:, sl]),
                }
            )
    return in_maps


def kernel(Mi, Mj, ri, rp, W, r):
    from concourse.bass_utils import run_bass_kernel_spmd

    global LAST_RESULTS
    nc = _get_nc()
    in_maps = _prep_inputs(Mi, Mj, ri, rp, W, r)
    # NTFF tracing needs the antenv.axon_hooks shim (test.py installs it);
    # without it the axon trace path raises, so force tracing off.
    trace = bool(os.environ.get("BASS_TRACE"))
    if "antenv.axon_hooks" not in sys.modules:
        trace = False
        os.environ["BASS_NEVER_TRACE"] = "1"
    res = run_bass_kernel_spmd(
        nc, in_maps, core_ids=list(range(NCORES)), trace=trace
    )
    LAST_RESULTS = res
    # Device margins are K^2-scaled partial sums over each NV shard; sum the
    # two shards per batch shard and undo the scale.
    margins = np.zeros(B, dtype=np.float64)
    for bi in range(NBSH):
        for n in range(NNSH):
            margins[bi * BS : (bi + 1) * BS] += res.results[bi * NNSH + n][
                "losses"
            ][0].astype(np.float64)
    margins /= float(K) * float(K)
    losses = np.maximum(margins + 1.0, 0.0)
    return np.float32(np.mean(losses))
